# revision 17
# baseline (speedup 1.0000x reference)
"""Trainium2 Bass kernel for nn_Block_29609504539559 (SE(3)-equivariant conv block).

Computation (matches the reference exactly):
  1. TensorProduct: outer products of the two dim-3 fields -> 18 extra channels
  2. SE3Convolution: 7^3 conv, 31 -> 13 channels, kernel = weight @ basis
  3. SE3BatchNorm: global (batch+spatial) stats per irrep group
  4. BiasRelu on the 2 scalar channels

Sharding: 8 cores = batch(4) x D-halves(2). Each core gets a halo-padded
input shard (host-prepared), computes its conv output block + partial BN
stats, all-reduces the tiny stats across cores, applies normalization
on-device, and writes its [13, 24, 48, 48] block.

Conv mapping to TensorE: block-Toeplitz along the first spatial axis.
  lhsT[(ich, u), (o, s)] = K[o, c, u - s, k2, k3]   (zero outside 0<=u-s<7)
  rhs[(ich, u), n=(y, z)] = padded input plane
  K-dim = 8 channels x 16-row window = 128, M = 13*8 = 104 (padded to 128),
  N = flat (y,z) chunks of <=480 (9 rows x 54-stride plane).
  Accumulate over 4 channel chunks x 49 (k2,k3) offsets = 196 matmuls/tile.
"""

import os
import sys
import types

import numpy as np

for _p in ("/opt/trn_rl_repo",):
    if _p not in sys.path and os.path.isdir(_p):
        sys.path.append(_p)

import ml_dtypes  # noqa: E402

import concourse.bacc as bacc  # noqa: E402
import concourse.mybir as mybir  # noqa: E402
import concourse.tile as tile  # noqa: E402
from concourse import bass_utils  # noqa: E402

F32 = mybir.dt.float32
NCORES = 8

# Problem shapes
B, D = 4, 48
C_IN, C_CONV, C_OUT = 13, 31, 13
KS, PAD = 7, 3
EPS = 1e-5

# Tiling
HALF = 24          # D-rows per core on the sharded axis
S = 8              # output rows (a-shifts) per window -> M = 13*8 = 104
U = 16             # input window rows on partitions (S + 6, padded to 16)
NW = 3             # windows per core (3*8 = 24 output rows)
CH = 8             # channels per K-chunk
NQ = 4             # channel chunks (4*8 = 32 >= 31)
PL = D + 2 * PAD   # 54 padded plane edge
D1P = D + PAD + 5  # 56: first-axis padding (3 before, 5 after for U=16)
NJ = 6             # n-chunks: 5 of 9 rows + 1 of 3 rows
M_VALID = C_OUT * S  # 104
PLANE = PL * PL    # 2916

_DT_NAME = os.environ.get("BASSK_DTYPE", "bf16")
if _DT_NAME == "bf16":
    DT_IN = mybir.dt.bfloat16
    NP_IN = ml_dtypes.bfloat16
elif _DT_NAME == "f32r":
    DT_IN = mybir.dt.float32r
    NP_IN = np.float32
elif _DT_NAME == "f32":
    DT_IN = mybir.dt.float32
    NP_IN = np.float32
else:
    raise ValueError(_DT_NAME)


def _install_ntff_shim():
    """antenv in this image lacks axon_hooks; provide it so trace=True works."""
    import antenv

    if "antenv.axon_hooks" in sys.modules:
        return
    mod = types.ModuleType("antenv.axon_hooks")
    state = {"hook": None}
    mod.set_axon_ntff_profile_hook = lambda h: state.__setitem__("hook", h)
    mod.get_axon_ntff_profile_hook = lambda: state["hook"]
    sys.modules["antenv.axon_hooks"] = mod
    antenv.axon_hooks = mod
    try:
        from trn_agent_boot.trn_boot import _ntff_profile_via_ctypes

        so = "/opt/axon/libaxon_pjrt.so"
        if os.path.exists(so):
            mod.set_axon_ntff_profile_hook(_ntff_profile_via_ctypes(so))
    except Exception:
        pass


_install_ntff_shim()


def _rj(j):
    return 9 if j < 5 else 3


def build_program():
    nc = bacc.Bacc(
        "TRN2", target_bir_lowering=False, debug=False, num_devices=NCORES
    )
    xin = nc.dram_tensor("xin", [NQ, NW, 128, PLANE], DT_IN, kind="ExternalInput")
    win = nc.dram_tensor("win", [NQ, 128, 49 * 128], DT_IN, kind="ExternalInput")
    sel = nc.dram_tensor("sel", [128, 128], F32, kind="ExternalInput")
    cvec = nc.dram_tensor("cvec", [128, 4], F32, kind="ExternalInput")
    out = nc.dram_tensor("out", [C_OUT, HALF, D, D], F32, kind="ExternalOutput")
    sdbg = nc.dram_tensor("sdbg", [128, 2], F32, kind="ExternalOutput")

    Ident = mybir.ActivationFunctionType.Identity
    Relu = mybir.ActivationFunctionType.Relu
    Sqrt = mybir.ActivationFunctionType.Sqrt

    with tile.TileContext(nc) as tc:
        with (
            tc.tile_pool(name="consts", bufs=1) as cpool,
            tc.tile_pool(name="wpool", bufs=1) as wpool,
            tc.tile_pool(name="xpool", bufs=1) as xpool,
            tc.tile_pool(name="ypool", bufs=1) as ypool,
            tc.tile_pool(name="stat", bufs=1) as stpool,
            tc.tile_pool(name="psum", bufs=4, space="PSUM") as pspool,
            tc.tile_pool(name="spsum", bufs=1, space="PSUM") as spspool,
            tc.tile_pool(name="scr", bufs=2) as scrpool,
            tc.tile_pool(name="opool", bufs=3) as opool,
            tc.tile_pool(name="dram", bufs=1, space="DRAM") as drpool,
        ):
            # ---- constants ----
            sel_t = cpool.tile([128, 128], F32, tag="sel")
            nc.sync.dma_start(sel_t[:], sel[:])
            cvec_t = cpool.tile([128, 4], F32, tag="cvec")
            nc.sync.dma_start(cvec_t[:], cvec[:])

            # ---- weights (block-Toeplitz lhsT tiles), resident ----
            w_t = []
            for q in range(NQ):
                wt = wpool.tile([128, 49 * 128], DT_IN, tag=f"w{q}")
                nc.sync.dma_start(wt[:], win[q])
                w_t.append(wt)

            # ---- input windows, resident ----
            x_t = []
            for q in range(NQ):
                xt = xpool.tile([128, NW, PLANE], DT_IN, tag=f"x{q}")
                for w in range(NW):
                    nc.sync.dma_start(xt[:, w, :], xin[q, w])
                x_t.append(xt)

            # ---- y storage + stats accumulators ----
            NT = NW * NJ  # 18 psum tiles
            y_all = ypool.tile([128, NT * 9 * 48], F32, tag="yall")
            sums1 = stpool.tile([128, NT], F32, tag="s1")
            sums2 = stpool.tile([128, NT], F32, tag="s2")
            nc.vector.memset(sums1[:], 0.0)
            nc.vector.memset(sums2[:], 0.0)

            # ---- conv: 18 psum tiles x 196 accumulated matmuls ----
            nwmax = int(os.environ.get("BASSK_NWMAX", str(NW)))
            for w in range(nwmax):
                for j in range(NJ):
                    rj = _rj(j)
                    nspan = (rj - 1) * PL + D
                    n0 = j * 9 * PL
                    t = w * NJ + j
                    pt = pspool.tile([128, 9 * PL], F32, tag="ps")
                    idx = 0
                    k2max = int(os.environ.get("BASSK_K2MAX", str(KS)))
                    last = NQ * k2max * KS - 1
                    for q in range(NQ):
                        for k2 in range(k2max):
                            for k3 in range(KS):
                                off = n0 + k2 * PL + k3
                                nc.tensor.matmul(
                                    pt[:, :nspan],
                                    w_t[q][:, (k2 * KS + k3) * 128:(k2 * KS + k3 + 1) * 128],
                                    x_t[q][:, w, off:off + nspan],
                                    start=(idx == 0),
                                    stop=(idx == last),
                                )
                                idx += 1
                    # valid region (excl. plane padding cols): [128, rj, 48]
                    pv = pt[:, :rj * PL].rearrange("p (r c) -> p r c", c=PL)[:, :, :D]
                    yv = y_all[:, t * 432: t * 432 + rj * D].rearrange(
                        "p (r c) -> p r c", c=D
                    )
                    statsv = os.environ.get("BASSK_STATSV", "v2")
                    if os.environ.get("BASSK_NO_STATS"):
                        nc.scalar.activation(yv, pv, Ident)
                    elif statsv == "v1":
                        # evacuate + per-partition sum on ACT
                        nc.scalar.activation(
                            yv, pv, Ident, accum_out=sums1[:, t:t + 1]
                        )
                        # squares + per-partition sum on DVE
                        sq = scrpool.tile([128, 9, D], F32, tag="sq")
                        nc.vector.tensor_tensor_reduce(
                            out=sq[:, :rj, :],
                            in0=yv,
                            in1=yv,
                            scale=1.0,
                            scalar=0.0,
                            op0=mybir.AluOpType.mult,
                            op1=mybir.AluOpType.add,
                            accum_out=sums2[:, t:t + 1],
                        )
                    elif statsv == "v2":
                        # evac+sum and square+sum, both on ACT
                        nc.scalar.activation(
                            yv, pv, Ident, accum_out=sums1[:, t:t + 1]
                        )
                        sq = scrpool.tile([128, 9, D], F32, tag="sq")
                        nc.scalar.activation(
                            sq[:, :rj, :], yv,
                            mybir.ActivationFunctionType.Square,
                            accum_out=sums2[:, t:t + 1],
                        )
                    else:  # v3: no fused accum at all
                        nc.scalar.activation(yv, pv, Ident)
                        nc.vector.reduce_sum(
                            sums1[:, t:t + 1], yv, axis=mybir.AxisListType.XY
                        )
                        sq = scrpool.tile([128, 9, D], F32, tag="sq")
                        nc.vector.tensor_mul(sq[:, :rj, :], yv, yv)
                        nc.vector.reduce_sum(
                            sums2[:, t:t + 1], sq[:, :rj, :],
                            axis=mybir.AxisListType.XY,
                        )

            stage = os.environ.get("BASSK_STAGE", "full")
            if stage in ("conv", "stats"):
                # raw conv output, skip normalize/apply
                for w in range(nwmax):
                    for j in range(NJ):
                        rj = _rj(j)
                        t = w * NJ + j
                        nc.sync.dma_start(
                            out[:, S * w:S * w + S, 9 * j:9 * j + rj, :],
                            y_all[:, t * 432: t * 432 + rj * D].rearrange(
                                "p (r c) -> p r c", c=D
                            )[0:M_VALID],
                        )

            # ---- stats: reduce 18 cols, partition-reduce via selector matmul ----
            if stage == "conv":
                tot = None  # skip everything below
            if stage != "conv":
              tot = stpool.tile([128, 2], F32, tag="tot")
            if stage != "conv":
              nc.vector.reduce_sum(tot[:, 0:1], sums1[:], axis=mybir.AxisListType.X)
              nc.vector.reduce_sum(tot[:, 1:2], sums2[:], axis=mybir.AxisListType.X)
            if stage != "conv":
              gsb = stpool.tile([128, 2], F32, tag="gsb")
              if os.environ.get("BASSK_NO_SELMM"):
                  nc.vector.tensor_copy(gsb[:], tot[:])
              else:
                  gps = spspool.tile([128, 2], F32, tag="gps")
                  nc.tensor.matmul(gps[:], sel_t[:], tot[:], start=True, stop=True)
                  nc.vector.tensor_copy(gsb[:], gps[:])

              # ---- all-reduce partial stats across the 8 cores ----
              bi = drpool.tile([128, 2], F32)
              bo = drpool.tile([128, 2], F32)
              nc.sync.dma_start(bi[:], gsb[:])
              if os.environ.get("BASSK_NO_CC"):
                  nc.sync.dma_start(bo[:], bi[:])
              else:
                  nc.gpsimd.collective_compute(
                      "AllReduce",
                      mybir.AluOpType.add,
                      replica_groups=[list(range(NCORES))],
                      ins=[bi.opt()],
                      outs=[bo.opt()],
                  )
              gstat = stpool.tile([128, 2], F32, tag="gstat")
              nc.sync.dma_start(gstat[:], bo[:])
              nc.sync.dma_start(sdbg[:], gstat[:])

            # ---- normalization params (per-partition [128,1]) ----
            # gstat[:,0] = mean (pre-mask), gstat[:,1] = E[y^2] per group
            if stage == "full":
              mask = cvec_t[:, 0:1]
              biasv = cvec_t[:, 1:2]
              mean = stpool.tile([128, 1], F32, tag="mean")
              nc.vector.tensor_mul(mean[:], gstat[:, 0:1], mask)
              msq = stpool.tile([128, 1], F32, tag="msq")
              nc.scalar.square(msq[:], mean[:])
              var = stpool.tile([128, 1], F32, tag="var")
              nc.vector.tensor_sub(var[:], gstat[:, 1:2], msq[:])
              nc.vector.tensor_scalar_add(var[:], var[:], EPS)
              sd = stpool.tile([128, 1], F32, tag="sd")
              nc.scalar.activation(sd[:], var[:], Sqrt)
              sc = stpool.tile([128, 1], F32, tag="sc")
              nc.vector.reciprocal(sc[:], sd[:])
              mts = stpool.tile([128, 1], F32, tag="mts")
              nc.vector.tensor_mul(mts[:], mean[:], sc[:])
              sh = stpool.tile([128, 1], F32, tag="sh")
              nc.vector.tensor_sub(sh[:], biasv, mts[:])

              # ---- apply affine (+relu on scalar rows) and write out ----
              for w in range(nwmax):
                for j in range(NJ):
                    rj = _rj(j)
                    t = w * NJ + j
                    yf = y_all[:, t * 432: t * 432 + rj * D]
                    of = opool.tile([128, 9 * D], F32, tag="of")
                    nc.scalar.activation(
                        of[:, :rj * D], yf, Ident, bias=sh[:], scale=sc[:]
                    )
                    nc.scalar.activation(
                        of[0:16, :rj * D], of[0:16, :rj * D], Relu
                    )
                    out_eng = (
                        nc.gpsimd
                        if os.environ.get("BASSK_OUTDMA") == "gpsimd"
                        else nc.sync
                    )
                    out_eng.dma_start(
                        out[:, S * w:S * w + S, 9 * j:9 * j + rj, :],
                        of[0:M_VALID, :rj * D].rearrange(
                            "p (r c) -> p r c", c=D
                        ),
                    )

    nc.compile()
    return nc


_NC = None


def _get_nc():
    global _NC
    if _NC is None:
        _NC = build_program()
    return _NC


def _host_prep(x, weight, basis, bias):
    x = np.ascontiguousarray(np.asarray(x, dtype=np.float32))
    weight = np.asarray(weight, dtype=np.float32)
    basis = np.asarray(basis, dtype=np.float32)
    bias = np.asarray(bias, dtype=np.float32)

    # tensor product channels
    v = x[:, 2:8].reshape(B, 2, 3, D, D, D)
    t = np.einsum("bfixyz,bfjxyz->bfijxyz", v, v).reshape(B, 18, D, D, D)

    # padded 32-channel input [B, 32, 56, 54, 54]
    xp = np.zeros((B, 32, D1P, PL, PL), dtype=np.float32)
    xp[:, :C_IN, PAD:PAD + D, PAD:PAD + D, PAD:PAD + D] = x
    xp[:, C_IN:C_CONV, PAD:PAD + D, PAD:PAD + D, PAD:PAD + D] = t

    # conv kernel K[o, c, dx, k2, k3], padded to 32 channels
    K = np.einsum("oib,bxyz->oixyz", weight, basis)
    Kp = np.zeros((C_OUT, 32, KS, KS, KS), dtype=np.float32)
    Kp[:, :C_CONV] = K
    # KQ[q, ich, dx, k23, o]
    KQ = Kp.reshape(C_OUT, NQ, CH, KS, 49).transpose(1, 2, 3, 4, 0)
    # lhsT[(q), (ich,u), k23, (o,s)]
    tmp = np.zeros((NQ, CH, U, 49, C_OUT, S), dtype=np.float32)
    for s in range(S):
        for dx in range(KS):
            tmp[:, :, s + dx, :, :, s] = KQ[:, :, dx, :, :]
    lhsT = np.zeros((NQ, 128, 49, 128), dtype=np.float32)
    lhsT[:, :, :, :M_VALID] = tmp.reshape(NQ, 128, 49, M_VALID)
    win_np = np.ascontiguousarray(
        lhsT.reshape(NQ, 128, 49 * 128).astype(NP_IN)
    )

    # selector [128, 128]: SEL[k, m] = inv_count[g] if g(k)==g(m) else 0
    NTOT = float(B * D * D * D)
    group = np.full(128, -1, dtype=np.int64)
    invc = np.zeros(128, dtype=np.float64)
    for o in range(C_OUT):
        if o < 2:
            g, cnt = o, NTOT
        elif o < 8:
            g, cnt = 2 + (o - 2) // 3, 3 * NTOT
        else:
            g, cnt = 4, 5 * NTOT
        for s in range(S):
            m = o * S + s
            group[m] = g
            invc[m] = 1.0 / cnt
    sel_np = np.zeros((128, 128), dtype=np.float32)
    for m in range(M_VALID):
        sel_np[:, m] = (group == group[m]) * invc[m]

    cvec_np = np.zeros((128, 4), dtype=np.float32)
    cvec_np[0:16, 0] = 1.0  # mask: scalar-channel rows
    cvec_np[0:8, 1] = bias[0]
    cvec_np[8:16, 1] = bias[1]

    # per-core rhs windows
    in_maps = []
    for core in range(NCORES):
        b, h = core // 2, core % 2
        d0 = HALF * h
        xin_np = np.empty((NQ, NW, 128, PLANE), dtype=NP_IN)
        for q in range(NQ):
            for w in range(NW):
                blk = xp[b, CH * q:CH * q + CH, d0 + S * w:d0 + S * w + U]
                xin_np[q, w] = blk.reshape(CH * U, PLANE).astype(NP_IN)
        in_maps.append(
            {
                "xin": xin_np,
                "win": win_np,
                "sel": sel_np,
                "cvec": cvec_np,
            }
        )
    return in_maps


def _run(x, weight, basis, bias, trace=False):
    nc = _get_nc()
    in_maps = _host_prep(x, weight, basis, bias)
    res = bass_utils.run_bass_kernel_spmd(
        nc, in_maps, core_ids=list(range(NCORES)), trace=trace
    )
    full = np.empty((B, C_OUT, D, D, D), dtype=np.float32)
    for core in range(NCORES):
        b, h = core // 2, core % 2
        d0 = HALF * h
        full[b, :, d0:d0 + HALF] = res.results[core]["out"]
    return full, res


def kernel(x, weight, basis, bias):
    out, _ = _run(x, weight, basis, bias, trace=False)
    return out


if __name__ == "__main__":
    rng = np.random.default_rng(0)
    x = rng.standard_normal((B, C_IN, D, D, D), dtype=np.float32)
    weight = rng.standard_normal((C_OUT, C_CONV, 75), dtype=np.float32) * 0.02
    basis = rng.standard_normal((75, KS, KS, KS), dtype=np.float32)
    bias = rng.standard_normal((2,), dtype=np.float32) * 0.1
    out = kernel(x, weight, basis, bias)
    print("out", out.shape, out.dtype, float(np.abs(out).mean()))
